# revision 1
# baseline (speedup 1.0000x reference)
"""Trainium2 Bass kernel v2 for nn_AdaptiveFourierTransformGateLayer.

Data-parallel over batch: 8 cores x 256 rows. Per core:

  Host prep: xw = x * fc_w (scale+layout only), reflection-fold over l:
    xe[b,c,l'] = xw[b,l',c] + xw[b,2048-l',c]   (l'=1..1023; l'=0 -> xw[b,0,c])
    xo[b,c,l'] = xw[b,l',c] - xw[b,2048-l',c]   (l'=0 -> 0)
    hm[b] = sum_c xw[b,1024,c]                  (midpoint row)
  This halves the DFT to 1024x1024 half-matrices (C even / S odd).
  fc_b is dropped: AC-bin column sums of the DFT are exactly zero.

  Device:
  A: c-tree reduction (DVE+GpSimd) xe/xo -> He/Ho [b,1024] f32->f32r,
     PE-transpose to HeT/HoT [l'-part, b].
  B: xrT[f,b] = Ch-chunks^T @ HeT (f32r matmuls, PSUM accumulate)
     + rank-1 midpoint term alt(f) x hm(b); xiT from Sh/HoT.
     4 rounds of 2 f-chunks (PSUM bank limit); round 0 overlaps stage A.
     Evac to fp16 xrT/xiT (+ negated xiTn).
  C: o1T[h,b] = relu(W1-chunks^T @ x*T + b1): transposed-dataflow fp16
     matmuls (stationary = weight chunk, moving = activations [128,256]),
     per-partition bias fused into the Relu evacuation. Plain 4-mm complex.
  D: layer 2 with psums q_A=o1r@W2r, q_B=o1i@W2i, q_i=o1i@W2r+o1r@W2i;
     amp = sqrt((q_A-q_B+b2r)^2 + (q_i+b2i)^2) -> ampT f32r.
  E: logits|noise = ampT-chunks @ wgn (f32r), noisy top-3 softmax -> gates.
"""
import sys
import types
import contextlib
import ctypes

import numpy as np

if "/opt/trn_rl_repo" not in sys.path:
    sys.path.insert(0, "/opt/trn_rl_repo")

# ---------------------------------------------------------------------------
# NTFF trace hook shim (only used when trace=True; harmless otherwise)
# ---------------------------------------------------------------------------


def _install_trace_shim():
    if "antenv.axon_hooks" in sys.modules:
        return
    so_path = "/opt/axon/libaxon_pjrt.so"

    def _mk():
        try:
            lib = ctypes.CDLL(so_path)
        except OSError:
            return None
        if not hasattr(lib, "axon_start_nrt_profile"):
            return None
        lib.axon_start_nrt_profile.argtypes = [
            ctypes.POINTER(ctypes.c_int64),
            ctypes.c_size_t,
        ]
        lib.axon_start_nrt_profile.restype = ctypes.c_int64
        lib.axon_stop_nrt_profile.argtypes = [ctypes.c_char_p]
        lib.axon_stop_nrt_profile.restype = ctypes.c_int64

        @contextlib.contextmanager
        def _hook(output_dir, device_ids):
            import jax

            jax.devices()
            if device_ids:
                ids = (ctypes.c_int64 * len(device_ids))(*device_ids)
                rc = lib.axon_start_nrt_profile(ids, len(device_ids))
            else:
                rc = lib.axon_start_nrt_profile(None, 0)
            if rc != 0:
                raise RuntimeError(f"axon_start_nrt_profile rc={rc}")
            try:
                yield
            finally:
                n = lib.axon_stop_nrt_profile(str(output_dir).encode())
                print(f"profile: {n} file(s) written to {output_dir}", file=sys.stderr)

        return _hook

    mod = types.ModuleType("antenv.axon_hooks")
    mod._hook = _mk()
    mod.get_axon_ntff_profile_hook = lambda: mod._hook
    mod.set_axon_ntff_profile_hook = lambda h: setattr(mod, "_hook", h)
    sys.modules["antenv.axon_hooks"] = mod
    try:
        import antenv

        antenv.axon_hooks = mod
    except ImportError:
        pass


_install_trace_shim()

import concourse.tile as tile  # noqa: E402
from concourse import bacc, mybir  # noqa: E402
from concourse.bass_utils import run_bass_kernel_spmd  # noqa: E402
from concourse.masks import make_identity  # noqa: E402

# ---------------------------------------------------------------------------
# Problem constants (hardcoded)
# ---------------------------------------------------------------------------
B = 2048
L = 2048
CH = 16
F = 1024  # num freqs (rfft bins 1..1024)
FH = 4096  # hidden
E = 88  # num experts
NOISE_EPS = 0.01
_DEBUG_DUMP = False
NCORES = 8
BL = B // NCORES  # 256 rows per core
F32R = mybir.dt.float32r
F32 = mybir.dt.float32
FP16 = mybir.dt.float16

ADD = mybir.AluOpType.add
MULT = mybir.AluOpType.mult
AF = mybir.ActivationFunctionType


def rnd11(x):
    """Round-to-nearest keeping 11 mantissa bits (hardware f32r rounding)."""
    a = np.ascontiguousarray(x, np.float32)
    ai = a.view(np.uint32)
    return ((ai + np.uint32(1 << 11)) & np.uint32(0xFFFFF000)).view(np.float32)


def _build_program(training: bool):
    nc = bacc.Bacc("TRN2", target_bir_lowering=False, debug=False, num_devices=NCORES)

    # [eo, bt, lc, p(b), c, l'-128] - host pre-tiled, fully contiguous chunks
    xeo_d = nc.dram_tensor("xeo", [2, 2, 8, 128, CH, 128], F32,
                           kind="ExternalInput").ap()
    # [half(fc 0-3 / 4-7), p(l'), kc, f-cols 512]
    chh_d = nc.dram_tensor("chh", [2, 128, 8, 512], F32R, kind="ExternalInput").ap()
    shh_d = nc.dram_tensor("shh", [2, 128, 8, 512], F32R, kind="ExternalInput").ap()
    # [hg, p(f), fc, h-cols 512]
    w1r_d = nc.dram_tensor("w1r", [8, 128, 8, 512], FP16, kind="ExternalInput").ap()
    w1i_d = nc.dram_tensor("w1i", [8, 128, 8, 512], FP16, kind="ExternalInput").ap()
    # [fp, p(h), hc, f-cols 256]
    w2r_d = nc.dram_tensor("w2r", [4, 128, 32, 256], FP16, kind="ExternalInput").ap()
    w2i_d = nc.dram_tensor("w2i", [4, 128, 32, 256], FP16, kind="ExternalInput").ap()
    # [p(f), fc, 256] - cols 0:88 gate, 128:216 noise
    wgn_d = nc.dram_tensor("wgn", [128, 8, 256], F32R, kind="ExternalInput").ap()
    hm_d = nc.dram_tensor("hmrow", [1, 256], F32R, kind="ExternalInput").ap()
    alt_d = nc.dram_tensor("altrow", [1, 128], F32R, kind="ExternalInput").ap()
    b1_d = nc.dram_tensor("b1all", [128, 64], F32, kind="ExternalInput").ap()  # r|i
    b2_d = nc.dram_tensor("b2all", [128, 16], F32, kind="ExternalInput").ap()  # r|i
    eps_d = nc.dram_tensor("eps", [128, 2, E], F32, kind="ExternalInput").ap()
    out_d = nc.dram_tensor("out", [BL, E], F32, kind="ExternalOutput").ap()
    if _DEBUG_DUMP:
        dbg_het = nc.dram_tensor("dbg_het", [128, 8, 256], F32R, kind="ExternalOutput").ap()
        dbg_hot = nc.dram_tensor("dbg_hot", [128, 8, 256], F32R, kind="ExternalOutput").ap()
        dbg_xr = nc.dram_tensor("dbg_xr", [128, 8, 256], FP16, kind="ExternalOutput").ap()
        dbg_xi = nc.dram_tensor("dbg_xi", [128, 8, 256], FP16, kind="ExternalOutput").ap()
        dbg_o1r = nc.dram_tensor("dbg_o1r", [128, 32, 256], FP16, kind="ExternalOutput").ap()
        dbg_amp = nc.dram_tensor("dbg_amp", [128, 8, 256], F32R, kind="ExternalOutput").ap()

    with tile.TileContext(nc) as tc:
        with tc.tile_pool(name="consts", bufs=1) as consts, \
             tc.tile_pool(name="xstream", bufs=3) as xstream, \
             tc.tile_pool(name="m16", bufs=4) as m16, \
             tc.tile_pool(name="h8", bufs=2) as h8, \
             tc.tile_pool(name="o16", bufs=2) as o16, \
             tc.tile_pool(name="acts", bufs=1) as acts, \
             tc.tile_pool(name="stage", bufs=4) as stage, \
             tc.tile_pool(name="ps", bufs=2, space="PSUM") as ps:

            ident = consts.tile([128, 128], F32, tag="ident")
            make_identity(nc, ident)
            ident_r = consts.tile([128, 128], F32R, tag="identr")
            nc.vector.tensor_copy(ident_r, ident)
            hm_sb = consts.tile([1, 256], F32R, tag="hm")
            nc.sync.dma_start(hm_sb, hm_d)
            alt_sb = consts.tile([1, 128], F32R, tag="alt")
            nc.sync.dma_start(alt_sb, alt_d)
            b1_sb = consts.tile([128, 64], F32, tag="b1")
            nc.sync.dma_start(b1_sb, b1_d)
            b2_sb = consts.tile([128, 16], F32, tag="b2")
            nc.sync.dma_start(b2_sb, b2_d)
            eps_sb = consts.tile([128, 2, E], F32, tag="eps")
            nc.sync.dma_start(eps_sb, eps_d)
            wgn_sb = consts.tile([128, 8, 256], F32R, tag="wgn")

            # persistent transposed activations
            HeT = h8.tile([128, 8, 256], F32R, tag="h8", name="HeT")
            HoT = h8.tile([128, 8, 256], F32R, tag="h8", name="HoT")
            xrT = acts.tile([128, 8, 256], FP16, tag="xrT")
            xiT = acts.tile([128, 8, 256], FP16, tag="xiT")
            xiTn = acts.tile([128, 8, 256], FP16, tag="xiTn")
            ampT = acts.tile([128, 8, 256], F32R, tag="ampT")

            # ---------------- Stage A + B ----------------
            scopeA = nc.named_scope("stageA_fc"); scopeA.__enter__()

            cs_sb = {}  # (mat, half) -> tile

            def a_chunk(eo, bt, lc, eng):
                """DMA one [128, 16, 128] chunk, tree-reduce over c, transpose."""
                xa = xstream.tile([128, CH, 128], F32, tag="big",
                                  name=f"x{eo}_{bt}_{lc}")
                nc.sync.dma_start(xa, xeo_d[eo][bt][lc])
                eng.tensor_tensor(xa[:, 0:8], xa[:, 0:8], xa[:, 8:16], op=ADD)
                eng.tensor_tensor(xa[:, 0:4], xa[:, 0:4], xa[:, 4:8], op=ADD)
                eng.tensor_tensor(xa[:, 0:2], xa[:, 0:2], xa[:, 2:4], op=ADD)
                hst = stage.tile([128, 128], F32, tag="hst", bufs=4,
                                 name=f"h{eo}_{bt}_{lc}")
                nc.vector.tensor_tensor(hst, xa[:, 0], xa[:, 1], op=ADD)
                pt = ps.tile([128, 128], F32, tag="pt", bufs=1,
                             name=f"pt{eo}_{bt}_{lc}")
                nc.tensor.transpose(pt, hst, ident)
                dst = HeT if eo == 0 else HoT
                if lc % 2 == 0:
                    nc.vector.tensor_copy(dst[:, lc, bt * 128:(bt + 1) * 128], pt)
                else:
                    nc.scalar.copy(dst[:, lc, bt * 128:(bt + 1) * 128], pt)

            def b_round(rnd, kcs):
                """Matmuls of B-round `rnd` (f-chunks 2rnd, 2rnd+1) for kcs.

                One accumulation group per PSUM bank: each fc gets its own
                [128, 256] psum tile (start=True in a bank wipes the bank).
                """
                half = rnd // 2
                ch = cs_sb[("c", half)]
                sh = cs_sb[("s", half)]
                for kc in kcs:
                    for j in range(2):
                        fc = rnd * 2 + j
                        cj = (fc % 4)
                        csl = slice(cj * 128, (cj + 1) * 128)
                        nc.tensor.matmul(psB[("r", fc)], ch[:, kc, csl],
                                         HeT[:, kc], start=(kc == 0), stop=False)
                        nc.tensor.matmul(psB[("i", fc)], sh[:, kc, csl],
                                         HoT[:, kc], start=(kc == 0),
                                         stop=(kc == 7))

            def b_finish(rnd):
                """Midpoint rank-1 term closes xr psums; evacuate to fp16."""
                for j in range(2):
                    fc = rnd * 2 + j
                    nc.tensor.matmul(psB[("r", fc)], alt_sb, hm_sb,
                                     start=False, stop=True)
                for j in range(2):
                    fc = rnd * 2 + j
                    nc.scalar.copy(xrT[:, fc], psB[("r", fc)])
                    nc.vector.tensor_copy(xiT[:, fc], psB[("i", fc)])
                    nc.vector.tensor_scalar(xiTn[:, fc], psB[("i", fc)], -1.0,
                                            None, op0=MULT)

            psB = {}
            for fc in (0, 1):
                psB[("r", fc)] = ps.tile([128, 256], F32, tag="acc", bufs=7,
                                         name=f"bxr{fc}")
                psB[("i", fc)] = ps.tile([128, 256], F32, tag="acc", bufs=7,
                                         name=f"bxi{fc}")

            # interleave x chunks with CS half DMAs; round-0 mms chase chunks
            npool = 0
            for lc in range(8):
                for eo in range(2):
                    for bt in range(2):
                        # ~1/3 of tree chunks on GpSimd, rest on DVE
                        npool += 1
                        eng = nc.vector  # DEBUG: gpsimd disabled
                        a_chunk(eo, bt, lc, eng)
                if lc == 1:
                    for m, d in (("c", chh_d), ("s", shh_d)):
                        t = m16.tile([128, 8, 512], F32R, tag="m16",
                                     name=f"cs{m}0")
                        nc.sync.dma_start(t, d[0])
                        cs_sb[(m, 0)] = t
                if lc == 4:
                    for m, d in (("c", chh_d), ("s", shh_d)):
                        t = m16.tile([128, 8, 512], F32R, tag="m16",
                                     name=f"cs{m}1")
                        nc.sync.dma_start(t, d[1])
                        cs_sb[(m, 1)] = t
                if lc >= 2:  # CS half0 available from lc=2 on
                    b_round(0, [lc - 2])
            b_round(0, [6, 7])
            b_finish(0)
            for rnd in range(1, 4):
                for fc in (rnd * 2, rnd * 2 + 1):
                    psB[("r", fc)] = ps.tile([128, 256], F32, tag="acc",
                                             bufs=7, name=f"bxr{fc}")
                    psB[("i", fc)] = ps.tile([128, 256], F32, tag="acc",
                                             bufs=7, name=f"bxi{fc}")
                b_round(rnd, range(8))
                b_finish(rnd)

            if _DEBUG_DUMP:
                nc.sync.dma_start(dbg_het, HeT)
                nc.sync.dma_start(dbg_hot, HoT)
                nc.sync.dma_start(dbg_xr, xrT)
                nc.sync.dma_start(dbg_xi, xiT)

            scopeA.__exit__(None, None, None)
            scopeC = nc.named_scope("stageC_l1"); scopeC.__enter__()

            o1rT = o16.tile([128, 32, 256], FP16, tag="o16", name="o1rT")
            o1iT = o16.tile([128, 32, 256], FP16, tag="o16", name="o1iT")

            for hg in range(8):
                w1r_sb = m16.tile([128, 8, 512], FP16, tag="m16", name=f"w1r{hg}")
                nc.sync.dma_start(w1r_sb, w1r_d[hg])
                w1i_sb = m16.tile([128, 8, 512], FP16, tag="m16", name=f"w1i{hg}")
                nc.sync.dma_start(w1i_sb, w1i_d[hg])
                for j in range(4):
                    hc = hg * 4 + j
                    p_r = ps.tile([128, 256], F32, tag="acc", bufs=7,
                                  name=f"cr{hc}")
                    p_i = ps.tile([128, 256], F32, tag="acc", bufs=7,
                                  name=f"ci{hc}")
                    hsl = slice(j * 128, (j + 1) * 128)
                    for fc in range(8):
                        f0 = fc == 0
                        fl = fc == 7
                        nc.tensor.matmul(p_r, w1r_sb[:, fc, hsl], xrT[:, fc],
                                         start=f0, stop=False)
                        nc.tensor.matmul(p_i, w1r_sb[:, fc, hsl], xiT[:, fc],
                                         start=f0, stop=False)
                        nc.tensor.matmul(p_i, w1i_sb[:, fc, hsl], xrT[:, fc],
                                         start=False, stop=fl)
                        nc.tensor.matmul(p_r, w1i_sb[:, fc, hsl], xiTn[:, fc],
                                         start=False, stop=fl)
                    nc.scalar.activation(o1rT[:, hc], p_r, AF.Relu,
                                         bias=b1_sb[:, hc:hc + 1])
                    nc.scalar.activation(o1iT[:, hc], p_i, AF.Relu,
                                         bias=b1_sb[:, 32 + hc:32 + hc + 1])

            scopeC.__exit__(None, None, None)
            scopeD = nc.named_scope("stageD_l2"); scopeD.__enter__()

            nc.sync.dma_start(wgn_sb, wgn_d)
            for fp in range(4):
                w2r_sb = m16.tile([128, 32, 256], FP16, tag="m16",
                                  name=f"w2r{fp}")
                nc.sync.dma_start(w2r_sb, w2r_d[fp])
                w2i_sb = m16.tile([128, 32, 256], FP16, tag="m16",
                                  name=f"w2i{fp}")
                nc.sync.dma_start(w2i_sb, w2i_d[fp])
                for fl in range(2):
                    fc = fp * 2 + fl
                    q_a = ps.tile([128, 256], F32, tag="acc", bufs=7,
                                  name=f"da{fc}")
                    q_b = ps.tile([128, 256], F32, tag="acc", bufs=7,
                                  name=f"db{fc}")
                    q_i = ps.tile([128, 256], F32, tag="acc", bufs=7,
                                  name=f"di{fc}")
                    fsl = slice(fl * 128, (fl + 1) * 128)
                    for hc in range(32):
                        h0 = hc == 0
                        hl = hc == 31
                        nc.tensor.matmul(q_a, w2r_sb[:, hc, fsl], o1rT[:, hc],
                                         start=h0, stop=hl)
                        nc.tensor.matmul(q_i, w2r_sb[:, hc, fsl], o1iT[:, hc],
                                         start=h0, stop=False)
                        nc.tensor.matmul(q_b, w2i_sb[:, hc, fsl], o1iT[:, hc],
                                         start=h0, stop=hl)
                        nc.tensor.matmul(q_i, w2i_sb[:, hc, fsl], o1rT[:, hc],
                                         start=False, stop=hl)
                    ta = stage.tile([128, 256], F32, tag="sq", bufs=4,
                                    name=f"ta{fc}")
                    nc.scalar.copy(ta, q_a)
                    sd = stage.tile([128, 256], F32, tag="sq", bufs=4,
                                    name=f"sd{fc}")
                    nc.vector.tensor_sub(sd, ta, q_b)
                    t1 = stage.tile([128, 256], F32, tag="sq", bufs=4,
                                    name=f"sqr{fc}")
                    nc.scalar.activation(t1, sd, AF.Square,
                                         bias=b2_sb[:, fc:fc + 1])
                    t2 = stage.tile([128, 256], F32, tag="sq", bufs=4,
                                    name=f"sqi{fc}")
                    nc.scalar.activation(t2, q_i, AF.Square,
                                         bias=b2_sb[:, 8 + fc:8 + fc + 1])
                    nc.vector.tensor_add(t1, t1, t2)
                    nc.scalar.activation(ampT[:, fc], t1, AF.Sqrt)

            if _DEBUG_DUMP:
                nc.sync.dma_start(dbg_o1r, o1rT)
                nc.sync.dma_start(dbg_amp, ampT)

            scopeD.__exit__(None, None, None)
            scopeE = nc.named_scope("stageE_gate"); scopeE.__enter__()

            out_v = out_d.rearrange("(bt p) e -> bt p e", bt=2)
            for bt in range(2):
                bs = slice(bt * 128, (bt + 1) * 128)
                pg = ps.tile([128, 256], F32, tag="acc", bufs=7, name=f"pg{bt}")
                for fc in range(8):
                    nc.tensor.matmul(pg, ampT[:, fc, bs], wgn_sb[:, fc],
                                     start=(fc == 0), stop=(fc == 7))
                logits = stage.tile([128, E], F32, tag="logits", bufs=2)
                if training:
                    stdn = stage.tile([128, E], F32, tag="stdn", bufs=2)
                    # softplus(z) = ln(1 + exp(z))
                    nc.scalar.activation(stdn, pg[:, 128:128 + E], AF.Exp)
                    nc.vector.tensor_scalar_add(stdn, stdn, 1.0)
                    nc.scalar.activation(stdn, stdn, AF.Ln)
                    # (softplus(z)+eps0)*eps
                    nc.vector.scalar_tensor_tensor(
                        stdn, stdn, float(NOISE_EPS), eps_sb[:, bt],
                        op0=ADD, op1=MULT)
                    nc.vector.tensor_add(logits, pg[:, 0:E], stdn)
                else:
                    nc.vector.tensor_copy(logits, pg[:, 0:E])
                top8 = stage.tile([128, 8], F32, tag="top8", bufs=2)
                nc.vector.max(top8, logits)
                negmax = stage.tile([128, 1], F32, tag="negmax", bufs=2)
                nc.vector.tensor_scalar(negmax, top8[:, 0:1], -1.0, None,
                                        op0=MULT)
                ex = stage.tile([128, E], F32, tag="ex", bufs=2)
                nc.scalar.activation(ex, logits, AF.Exp, bias=negmax)
                msk = stage.tile([128, E], F32, tag="msk", bufs=2)
                nc.vector.tensor_scalar(msk, logits, top8[:, 2:3], None,
                                        op0=mybir.AluOpType.is_ge)
                nc.vector.tensor_mul(ex, ex, msk)
                ssum = stage.tile([128, 1], F32, tag="ssum", bufs=2)
                nc.vector.reduce_sum(out=ssum, in_=ex, axis=mybir.AxisListType.X)
                rinv = stage.tile([128, 1], F32, tag="rinv", bufs=2)
                nc.vector.reciprocal(rinv, ssum)
                gates = stage.tile([128, E], F32, tag="gates", bufs=2)
                nc.vector.tensor_scalar(gates, ex, rinv, None, op0=MULT)
                nc.sync.dma_start(out_v[bt], gates)

            scopeE.__exit__(None, None, None)

    nc.compile()
    return nc


_PROGRAM_CACHE = {}


def _get_program(training: bool):
    key = bool(training)
    if key not in _PROGRAM_CACHE:
        _PROGRAM_CACHE[key] = _build_program(key)
    return _PROGRAM_CACHE[key]


def _prep_inputs(x, fc_w, fc_b, w1, b1, w2, b2, w_gate, w_noise, eps):
    f32 = np.float32
    f16 = np.float16

    # ---- weights/constants shared by all cores ----
    ll = np.arange(1, F, dtype=np.int64)[:, None]  # l' = 1..1023
    ff = np.arange(1, F + 1, dtype=np.int64)[None, :]
    ang = 2.0 * np.pi * ((ll * ff) % L).astype(np.float64) / L
    scale = 1.0 / np.sqrt(L)
    Ch = np.empty((F, F), np.float64)
    Sh = np.empty((F, F), np.float64)
    Ch[0, :] = scale
    Ch[1:, :] = np.cos(ang) * scale
    Sh[0, :] = 0.0
    Sh[1:, :] = -np.sin(ang) * scale
    # [half, p(l'), kc, 512]
    chh = rnd11(Ch.astype(f32)).reshape(8, 128, 2, 512).transpose(2, 1, 0, 3)
    shh = rnd11(Sh.astype(f32)).reshape(8, 128, 2, 512).transpose(2, 1, 0, 3)

    # midpoint row: C[1024, f] = (-1)^f * scale, f = p+1 within each chunk
    p = np.arange(128)
    alt = (np.where((p + 1) % 2 == 0, 1.0, -1.0) * scale).astype(f32)
    alt = rnd11(alt).reshape(1, 128)

    w1r = np.asarray(w1[0], f32).astype(f16).reshape(8, 128, 8, 512).transpose(2, 1, 0, 3)
    w1i = np.asarray(w1[1], f32).astype(f16).reshape(8, 128, 8, 512).transpose(2, 1, 0, 3)
    w2r = np.asarray(w2[0], f32).astype(f16).reshape(32, 128, 4, 256).transpose(2, 1, 0, 3)
    w2i = np.asarray(w2[1], f32).astype(f16).reshape(32, 128, 4, 256).transpose(2, 1, 0, 3)

    wgn = np.zeros((F, 256), f32)
    wgn[:, 0:E] = np.asarray(w_gate, f32)
    wgn[:, 128:128 + E] = np.asarray(w_noise, f32)
    wgn = rnd11(wgn).reshape(8, 128, 256).transpose(1, 0, 2)

    b1all = np.zeros((128, 64), f32)
    b1all[:, 0:32] = np.asarray(b1[0], f32).reshape(32, 128).T
    b1all[:, 32:64] = np.asarray(b1[1], f32).reshape(32, 128).T
    b2all = np.zeros((128, 16), f32)
    b2all[:, 0:8] = np.asarray(b2[0], f32).reshape(8, 128).T
    b2all[:, 8:16] = np.asarray(b2[1], f32).reshape(8, 128).T

    common = {
        "chh": np.ascontiguousarray(chh),
        "shh": np.ascontiguousarray(shh),
        "w1r": np.ascontiguousarray(w1r),
        "w1i": np.ascontiguousarray(w1i),
        "w2r": np.ascontiguousarray(w2r),
        "w2i": np.ascontiguousarray(w2i),
        "wgn": np.ascontiguousarray(wgn),
        "altrow": alt,
        "b1all": b1all,
        "b2all": b2all,
    }

    # ---- per-core data ----
    x = np.asarray(x, f32)
    fcw = np.asarray(fc_w, f32).reshape(CH)
    eps = np.asarray(eps, f32)

    in_maps = []
    for i in range(NCORES):
        xs = x[i * BL:(i + 1) * BL]  # [256, 2048, 16]
        # xw[b, c, l] = x[b, l, c] * fc_w[c] (scale folded host-side)
        xw = xs.transpose(0, 2, 1) * fcw[None, :, None]  # [256, 16, 2048]
        xe = np.empty((BL, CH, F), f32)
        xo = np.empty((BL, CH, F), f32)
        xe[:, :, 0] = xw[:, :, 0]
        xo[:, :, 0] = 0.0
        fwd = xw[:, :, 1:1024]
        rev = xw[:, :, 2047:1024:-1]
        xe[:, :, 1:1024] = fwd + rev
        xo[:, :, 1:1024] = fwd - rev
        hm = xw[:, :, 1024].sum(axis=1)  # [256]
        # tile: [eo, b, c, l'] -> [eo, bt, lc, p, c, 128]
        xeo = np.stack([xe, xo])  # [2, 256, 16, 1024]
        xeo = xeo.reshape(2, 2, 128, CH, 8, 128).transpose(0, 1, 4, 2, 3, 5)
        sh = dict(common)
        sh["xeo"] = np.ascontiguousarray(xeo)
        sh["hmrow"] = rnd11(hm).reshape(1, 256)
        esh = eps[i * BL:(i + 1) * BL]  # [256, E]
        sh["eps"] = np.ascontiguousarray(esh.reshape(2, 128, E).transpose(1, 0, 2))
        in_maps.append(sh)
    return in_maps


def run(inputs, trace=False):
    """Returns (gates [B, E] float32, BassKernelResults)."""
    x = np.asarray(inputs["x"], np.float32)
    fc_w = np.asarray(inputs["fc_w"], np.float32)
    fc_b = np.asarray(inputs["fc_b"], np.float32)
    w1 = np.asarray(inputs["w1"], np.float32)
    b1 = np.asarray(inputs["b1"], np.float32)
    w2 = np.asarray(inputs["w2"], np.float32)
    b2 = np.asarray(inputs["b2"], np.float32)
    w_gate = np.asarray(inputs["w_gate"], np.float32)
    w_noise = np.asarray(inputs["w_noise"], np.float32)
    eps = np.asarray(inputs["eps"], np.float32)
    training = bool(int(np.asarray(inputs.get("training", 1))))

    nc = _get_program(training)
    in_maps = _prep_inputs(x, fc_w, fc_b, w1, b1, w2, b2, w_gate, w_noise, eps)
    res = run_bass_kernel_spmd(
        nc, in_maps, core_ids=list(range(NCORES)), trace=trace,
    )
    gates = np.concatenate([r["out"] for r in res.results], axis=0)
    return gates.astype(np.float32), res


def kernel(**inputs):
    gates, _ = run(inputs, trace=False)
    return gates



# revision 8
# speedup vs baseline: 1.2676x; 1.2676x over previous
"""Trainium2 Bass kernel v3 for nn_AdaptiveFourierTransformGateLayer.

Data-parallel over batch: 8 cores x 256 rows. Per core:

  Host prep: xw = x * fc_w (scale+layout only), reflection-fold over l:
    xe[b,c,l'] = xw[b,l',c] + xw[b,2048-l',c]   (l'=1..1023; l'=0 -> xw[b,0,c])
    xo[b,c,l'] = xw[b,l',c] - xw[b,2048-l',c]   (l'=0 -> 0)
    hm[b] = sum_c xw[b,1024,c]                  (midpoint row)
  Folding halves the DFT to 1024x1024 half-matrices (C even / S odd).
  fc_b is dropped: AC-bin column sums of the DFT are exactly zero.
  Everything streamed in fp16, laid out [l'-part, c, b] so the channel
  tree-reduce lands directly in DFT-ready [l', b] layout (no transposes).

  Device:
  A: c-tree reduction (DVE, fp16 2x mode) -> HeT/HoT [l'-part, b].
  B: DFT chase: per f-chunk PSUM bank holds xr | xi halves; fp16 matmuls
     Ch-chunk^T @ HeT / Sh-chunk^T @ HoT accumulate as l'-chunks arrive.
     Midpoint rank-1 term alt(f) x hm(b) closes xr. 7 banks chase, f-chunk
     7 runs as a second wave after bank 0 evacuates. Evac to fp16
     xr/xi/xs (xs = xr+xi for Karatsuba).
  C: layer 1 via 3-matmul Karatsuba complex product:
       m1 = (xr+xi)@W1r, m2 = xi@(W1r+W1i), m3 = xr@(W1i-W1r)
       o1r = relu(m1-m2+b1r), o1i = relu(m1+m3+b1i), o1s = o1r+o1i
     m1|m2 share a PSUM bank, m3 in a second bank. Transposed dataflow
     (stationary = weight chunk, moving = activations [128,256]).
  D: layer 2 same Karatsuba shape; amp = sqrt((m1-m2+b2r)^2+(m1+m3+b2i)^2)
     -> ampT f32r. Gate matmuls (ampT @ wgn, f32r) chased per f-chunk.
  E: noisy top-3 softmax -> gates (small DVE/Act chain only).
"""
import sys
import types
import contextlib
import ctypes

import numpy as np

if "/opt/trn_rl_repo" not in sys.path:
    sys.path.insert(0, "/opt/trn_rl_repo")

# ---------------------------------------------------------------------------
# NTFF trace hook shim (only used when trace=True; harmless otherwise)
# ---------------------------------------------------------------------------


def _install_trace_shim():
    if "antenv.axon_hooks" in sys.modules:
        return
    so_path = "/opt/axon/libaxon_pjrt.so"

    def _mk():
        try:
            lib = ctypes.CDLL(so_path)
        except OSError:
            return None
        if not hasattr(lib, "axon_start_nrt_profile"):
            return None
        lib.axon_start_nrt_profile.argtypes = [
            ctypes.POINTER(ctypes.c_int64),
            ctypes.c_size_t,
        ]
        lib.axon_start_nrt_profile.restype = ctypes.c_int64
        lib.axon_stop_nrt_profile.argtypes = [ctypes.c_char_p]
        lib.axon_stop_nrt_profile.restype = ctypes.c_int64

        @contextlib.contextmanager
        def _hook(output_dir, device_ids):
            import jax

            jax.devices()
            if device_ids:
                ids = (ctypes.c_int64 * len(device_ids))(*device_ids)
                rc = lib.axon_start_nrt_profile(ids, len(device_ids))
            else:
                rc = lib.axon_start_nrt_profile(None, 0)
            if rc != 0:
                raise RuntimeError(f"axon_start_nrt_profile rc={rc}")
            try:
                yield
            finally:
                n = lib.axon_stop_nrt_profile(str(output_dir).encode())
                print(f"profile: {n} file(s) written to {output_dir}", file=sys.stderr)

        return _hook

    mod = types.ModuleType("antenv.axon_hooks")
    mod._hook = _mk()
    mod.get_axon_ntff_profile_hook = lambda: mod._hook
    mod.set_axon_ntff_profile_hook = lambda h: setattr(mod, "_hook", h)
    sys.modules["antenv.axon_hooks"] = mod
    try:
        import antenv

        antenv.axon_hooks = mod
    except ImportError:
        pass


_install_trace_shim()

import concourse.tile as tile  # noqa: E402
from concourse import bacc, mybir  # noqa: E402
from concourse.bass_utils import run_bass_kernel_spmd  # noqa: E402

# ---------------------------------------------------------------------------
# Problem constants (hardcoded)
# ---------------------------------------------------------------------------
B = 2048
L = 2048
CH = 16
F = 1024  # num freqs (rfft bins 1..1024)
FH = 4096  # hidden
E = 88  # num experts
NOISE_EPS = 0.01
_DEBUG_DUMP = False
NCORES = 8
BL = B // NCORES  # 256 rows per core
F32R = mybir.dt.float32r
F32 = mybir.dt.float32
FP16 = mybir.dt.float16

KARA1 = False  # Karatsuba in layer 1 (off: fp16 weight combos + relu
               # boundary flips cost 3x accuracy; plain 4-matmul instead)
KARA2 = True  # ... in layer 2

ADD = mybir.AluOpType.add
MULT = mybir.AluOpType.mult
AF = mybir.ActivationFunctionType


def rnd11(x):
    """Round-to-nearest keeping 11 mantissa bits (hardware f32r rounding)."""
    a = np.ascontiguousarray(x, np.float32)
    ai = a.view(np.uint32)
    return ((ai + np.uint32(1 << 11)) & np.uint32(0xFFFFF000)).view(np.float32)


def _build_program(training: bool):
    nc = bacc.Bacc("TRN2", target_bir_lowering=False, debug=False, num_devices=NCORES)

    # [eo, lc, p(l'), c, b] - host pre-transposed so tree-reduce -> [l', b]
    xeo_d = nc.dram_tensor("xeo", [2, 8, 128, CH, 256], FP16,
                           kind="ExternalInput").ap()
    # [p(l'), kc, fc, 128 f-cols]
    chs_d = nc.dram_tensor("chs", [128, 8, 8, 128], FP16, kind="ExternalInput").ap()
    shs_d = nc.dram_tensor("shs", [128, 8, 8, 128], FP16, kind="ExternalInput").ap()
    # [hg, p(f), fc, h-cols 512]
    k1_d = nc.dram_tensor("k1", [8, 128, 8, 512], FP16, kind="ExternalInput").ap()
    k2_d = nc.dram_tensor("k2", [8, 128, 8, 512], FP16, kind="ExternalInput").ap()
    k3_d = nc.dram_tensor("k3", [8, 128, 8, 512], FP16, kind="ExternalInput").ap()
    # [fp, p(h), hc, f-cols 256]
    m1_d = nc.dram_tensor("m1", [4, 128, 32, 256], FP16, kind="ExternalInput").ap()
    m2_d = nc.dram_tensor("m2", [4, 128, 32, 256], FP16, kind="ExternalInput").ap()
    m3_d = nc.dram_tensor("m3", [4, 128, 32, 256], FP16, kind="ExternalInput").ap()
    # [p(f), fc, 256] - cols 0:88 gate, 128:216 noise
    wgn_d = nc.dram_tensor("wgn", [128, 8, 256], F32R, kind="ExternalInput").ap()
    hm_d = nc.dram_tensor("hmrow", [1, 256], FP16, kind="ExternalInput").ap()
    alt_d = nc.dram_tensor("altrow", [1, 128], FP16, kind="ExternalInput").ap()
    b1_d = nc.dram_tensor("b1all", [128, 64], F32, kind="ExternalInput").ap()  # r|i
    b2_d = nc.dram_tensor("b2all", [128, 16], F32, kind="ExternalInput").ap()  # r|i
    eps_d = nc.dram_tensor("eps", [128, 2, E], F32, kind="ExternalInput").ap()
    out_d = nc.dram_tensor("out", [BL, E], F32, kind="ExternalOutput").ap()
    if _DEBUG_DUMP:
        dbg_het = nc.dram_tensor("dbg_het", [128, 8, 256], FP16, kind="ExternalOutput").ap()
        dbg_hot = nc.dram_tensor("dbg_hot", [128, 8, 256], FP16, kind="ExternalOutput").ap()
        dbg_xr = nc.dram_tensor("dbg_xr", [128, 8, 256], FP16, kind="ExternalOutput").ap()
        dbg_xi = nc.dram_tensor("dbg_xi", [128, 8, 256], FP16, kind="ExternalOutput").ap()
        dbg_o1r = nc.dram_tensor("dbg_o1r", [128, 32, 256], FP16, kind="ExternalOutput").ap()
        dbg_amp = nc.dram_tensor("dbg_amp", [128, 8, 256], F32R, kind="ExternalOutput").ap()

    with tile.TileContext(nc) as tc:
        with tc.tile_pool(name="consts", bufs=1) as consts, \
             tc.tile_pool(name="xstream", bufs=3) as xstream, \
             tc.tile_pool(name="wring", bufs=5) as wring, \
             tc.tile_pool(name="h8", bufs=1) as h8, \
             tc.tile_pool(name="acts", bufs=1) as acts, \
             tc.tile_pool(name="o16", bufs=1) as o16, \
             tc.tile_pool(name="stage", bufs=4) as stage, \
             tc.tile_pool(name="ps", bufs=1, space="PSUM") as ps:

            hm_sb = consts.tile([1, 256], FP16, tag="hm")
            nc.sync.dma_start(hm_sb, hm_d)
            alt_sb = consts.tile([1, 128], FP16, tag="alt")
            nc.sync.dma_start(alt_sb, alt_d)
            b1_sb = consts.tile([128, 64], F32, tag="b1")
            nc.sync.dma_start(b1_sb, b1_d)
            b2_sb = consts.tile([128, 16], F32, tag="b2")
            nc.sync.dma_start(b2_sb, b2_d)
            eps_sb = consts.tile([128, 2, E], F32, tag="eps")
            nc.sync.dma_start(eps_sb, eps_d)
            wgn_sb = consts.tile([128, 8, 256], F32R, tag="wgn")

            # DFT half-matrices, first slots of the streaming weight ring
            chs = wring.tile([128, 8, 8, 128], FP16, tag="w", name="chs")
            nc.sync.dma_start(chs, chs_d)
            shs = wring.tile([128, 8, 8, 128], FP16, tag="w", name="shs")
            nc.sync.dma_start(shs, shs_d)

            # persistent activations
            HeT = h8.tile([128, 8, 256], FP16, tag="he", name="HeT")
            HoT = h8.tile([128, 8, 256], FP16, tag="ho", name="HoT")
            xrT = acts.tile([128, 8, 256], FP16, tag="xr")
            xiT = acts.tile([128, 8, 256], FP16, tag="xi")
            # 3rd stream: xs = xr+xi (Karatsuba) or xin = -xi (plain)
            x3T = acts.tile([128, 8, 256], FP16, tag="x3")
            ampT = acts.tile([128, 8, 256], F32R, tag="amp")
            o1rT = o16.tile([128, 32, 256], FP16, tag="o1r", name="o1rT")
            o1iT = o16.tile([128, 32, 256], FP16, tag="o1i", name="o1iT")
            o1sT = None
            if KARA2:
                o1sT = o16.tile([128, 32, 256], FP16, tag="o1s", name="o1sT")

            # ---------------- Stage A + B ----------------
            scopeA = nc.named_scope("stageA_dft"); scopeA.__enter__()

            psB = {}

            def bbank(fc):
                psB[fc] = ps.tile([128, 512], F32, tag="bank", bufs=7,
                                  name=f"B{fc}")

            def b_mms(fc, lc):
                nc.tensor.matmul(psB[fc][:, 0:256], chs[:, lc, fc], HeT[:, lc],
                                 start=(lc == 0), stop=False)
                nc.tensor.matmul(psB[fc][:, 256:512], shs[:, lc, fc],
                                 HoT[:, lc], start=False, stop=(lc == 7))

            def b_mid(fc):
                nc.tensor.matmul(psB[fc][:, 0:256], alt_sb, hm_sb,
                                 start=False, stop=True)

            def b_evac(fc):
                nc.scalar.copy(xrT[:, fc], psB[fc][:, 0:256])
                nc.scalar.copy(xiT[:, fc], psB[fc][:, 256:512])
                if KARA1:
                    # one-PSUM-operand rule: xr is already in SBUF (fp16)
                    nc.vector.tensor_tensor(x3T[:, fc], xrT[:, fc],
                                            psB[fc][:, 256:512], op=ADD)
                else:
                    nc.vector.tensor_scalar(x3T[:, fc], psB[fc][:, 256:512],
                                            -1.0, None, op0=MULT)

            for fc in range(7):
                bbank(fc)
            for lc in range(8):
                for eo in range(2):
                    xa = xstream.tile([128, CH, 256], FP16, tag="big",
                                      name=f"x{eo}_{lc}")
                    nc.sync.dma_start(xa, xeo_d[eo][lc])
                    nc.vector.tensor_tensor(xa[:, 0:8], xa[:, 0:8],
                                            xa[:, 8:16], op=ADD)
                    nc.vector.tensor_tensor(xa[:, 0:4], xa[:, 0:4],
                                            xa[:, 4:8], op=ADD)
                    nc.vector.tensor_tensor(xa[:, 0:2], xa[:, 0:2],
                                            xa[:, 2:4], op=ADD)
                    dst = HeT if eo == 0 else HoT
                    nc.vector.tensor_tensor(dst[:, lc], xa[:, 0], xa[:, 1],
                                            op=ADD)
                for fc in range(7):
                    b_mms(fc, lc)
            for fc in range(7):
                b_mid(fc)
            b_evac(0)
            # wave 2: f-chunk 7 reuses bank slot of f-chunk 0
            bbank(7)
            for lc in range(8):
                b_mms(7, lc)
            b_mid(7)
            for fc in range(1, 8):
                b_evac(fc)

            if _DEBUG_DUMP:
                nc.sync.dma_start(dbg_het, HeT)
                nc.sync.dma_start(dbg_hot, HoT)
                nc.sync.dma_start(dbg_xr, xrT)
                nc.sync.dma_start(dbg_xi, xiT)

            scopeA.__exit__(None, None, None)
            scopeC = nc.named_scope("stageC_l1"); scopeC.__enter__()

            nc.sync.dma_start(wgn_sb, wgn_d)

            for hg in range(8):
                k1 = wring.tile([128, 8, 512], FP16, tag="w", name=f"k1g{hg}")
                nc.sync.dma_start(k1, k1_d[hg])
                k2 = wring.tile([128, 8, 512], FP16, tag="w", name=f"k2g{hg}")
                nc.sync.dma_start(k2, k2_d[hg])
                k3 = None
                if KARA1:
                    k3 = wring.tile([128, 8, 512], FP16, tag="w",
                                    name=f"k3g{hg}")
                    nc.sync.dma_start(k3, k3_d[hg])
                for j in range(4):
                    hc = hg * 4 + j
                    bA = ps.tile([128, 512], F32, tag="bank", bufs=7,
                                 name=f"cA{hc}")
                    bB = None
                    if KARA1:
                        bB = ps.tile([128, 512], F32, tag="bank", bufs=7,
                                     name=f"cB{hc}")
                    hsl = slice(j * 128, (j + 1) * 128)
                    for fc in range(8):
                        f0 = fc == 0
                        fl_ = fc == 7
                        if KARA1:
                            # m1 = (xr+xi)@W1r ; m2 = xi@(W1r+W1i)
                            # m3 = xr@(W1i-W1r)
                            nc.tensor.matmul(bA[:, 0:256], k1[:, fc, hsl],
                                             x3T[:, fc], start=f0, stop=fl_)
                            nc.tensor.matmul(bA[:, 256:512], k2[:, fc, hsl],
                                             xiT[:, fc], start=False, stop=fl_)
                            nc.tensor.matmul(bB[:, 0:256], k3[:, fc, hsl],
                                             xrT[:, fc], start=f0, stop=fl_)
                        else:
                            # o1r = xr@W1r + (-xi)@W1i ; o1i = xi@W1r + xr@W1i
                            nc.tensor.matmul(bA[:, 0:256], k1[:, fc, hsl],
                                             xrT[:, fc], start=f0, stop=False)
                            nc.tensor.matmul(bA[:, 0:256], k2[:, fc, hsl],
                                             x3T[:, fc], start=False, stop=fl_)
                            nc.tensor.matmul(bA[:, 256:512], k1[:, fc, hsl],
                                             xiT[:, fc], start=False, stop=False)
                            nc.tensor.matmul(bA[:, 256:512], k2[:, fc, hsl],
                                             xrT[:, fc], start=False, stop=fl_)
                    if KARA1:
                        # one-PSUM-operand rule: stage m1 through SBUF
                        tm = stage.tile([128, 256], F32, tag="d", bufs=4,
                                        name=f"tm_{hc}")
                        nc.scalar.copy(tm, bA[:, 0:256])
                        d1 = stage.tile([128, 256], F32, tag="d", bufs=4,
                                        name=f"d1_{hc}")
                        nc.vector.tensor_sub(d1, tm, bA[:, 256:512])
                        nc.scalar.activation(o1rT[:, hc], d1, AF.Relu,
                                             bias=b1_sb[:, hc:hc + 1])
                        d2 = stage.tile([128, 256], F32, tag="d", bufs=4,
                                        name=f"d2_{hc}")
                        nc.vector.tensor_add(d2, tm, bB[:, 0:256])
                        nc.scalar.activation(o1iT[:, hc], d2, AF.Relu,
                                             bias=b1_sb[:, 32 + hc:33 + hc])
                    else:
                        nc.scalar.activation(o1rT[:, hc], bA[:, 0:256], AF.Relu,
                                             bias=b1_sb[:, hc:hc + 1])
                        nc.scalar.activation(o1iT[:, hc], bA[:, 256:512],
                                             AF.Relu,
                                             bias=b1_sb[:, 32 + hc:33 + hc])
                    if KARA2:
                        nc.vector.tensor_tensor(o1sT[:, hc], o1rT[:, hc],
                                                o1iT[:, hc], op=ADD)

            scopeC.__exit__(None, None, None)
            scopeD = nc.named_scope("stageD_l2"); scopeD.__enter__()

            pgt = ps.tile([128, 2, 256], F32, tag="pg", bufs=1, name="pg")
            pg = [pgt[:, 0], pgt[:, 1]]

            for fp in range(4):
                m1 = wring.tile([128, 32, 256], FP16, tag="w", name=f"m1g{fp}")
                nc.sync.dma_start(m1, m1_d[fp])
                m2 = wring.tile([128, 32, 256], FP16, tag="w", name=f"m2g{fp}")
                nc.sync.dma_start(m2, m2_d[fp])
                m3 = None
                if KARA2:
                    m3 = wring.tile([128, 32, 256], FP16, tag="w",
                                    name=f"m3g{fp}")
                    nc.sync.dma_start(m3, m3_d[fp])
                for fl in range(2):
                    fc = fp * 2 + fl
                    bA = ps.tile([128, 512], F32, tag="bank", bufs=7,
                                 name=f"dA{fc}")
                    bB = ps.tile([128, 512], F32, tag="bank", bufs=7,
                                 name=f"dB{fc}")
                    fsl = slice(fl * 128, (fl + 1) * 128)
                    for hc in range(32):
                        h0 = hc == 0
                        hl = hc == 31
                        if KARA2:
                            # m1 = o1s@W2r ; m2 = o1i@(W2r+W2i)
                            # m3 = o1r@(W2i-W2r)
                            nc.tensor.matmul(bA[:, 0:256], m1[:, hc, fsl],
                                             o1sT[:, hc], start=h0, stop=hl)
                            nc.tensor.matmul(bA[:, 256:512], m2[:, hc, fsl],
                                             o1iT[:, hc], start=False, stop=hl)
                            nc.tensor.matmul(bB[:, 0:256], m3[:, hc, fsl],
                                             o1rT[:, hc], start=h0, stop=hl)
                        else:
                            # qa|qb in bA, qi in bB:
                            # o2r = qa - qb ; o2i = qi
                            nc.tensor.matmul(bA[:, 0:256], m1[:, hc, fsl],
                                             o1rT[:, hc], start=h0, stop=hl)
                            nc.tensor.matmul(bA[:, 256:512], m2[:, hc, fsl],
                                             o1iT[:, hc], start=False, stop=hl)
                            nc.tensor.matmul(bB[:, 0:256], m1[:, hc, fsl],
                                             o1iT[:, hc], start=h0, stop=False)
                            nc.tensor.matmul(bB[:, 0:256], m2[:, hc, fsl],
                                             o1rT[:, hc], start=False, stop=hl)
                    # one-PSUM-operand rule: stage m1 through SBUF first
                    tm = stage.tile([128, 256], F32, tag="d", bufs=4,
                                    name=f"tm{fc}")
                    nc.scalar.copy(tm, bA[:, 0:256])
                    sd = stage.tile([128, 256], F32, tag="d", bufs=4,
                                    name=f"sd{fc}")
                    nc.vector.tensor_sub(sd, tm, bA[:, 256:512])
                    t1 = stage.tile([128, 256], F32, tag="d", bufs=4,
                                    name=f"sqr{fc}")
                    nc.scalar.activation(t1, sd, AF.Square,
                                         bias=b2_sb[:, fc:fc + 1])
                    t2 = stage.tile([128, 256], F32, tag="d", bufs=4,
                                    name=f"sqi{fc}")
                    if KARA2:
                        si = stage.tile([128, 256], F32, tag="d", bufs=4,
                                        name=f"si{fc}")
                        nc.vector.tensor_add(si, tm, bB[:, 0:256])
                        nc.scalar.activation(t2, si, AF.Square,
                                             bias=b2_sb[:, 8 + fc:9 + fc])
                    else:
                        nc.scalar.activation(t2, bB[:, 0:256], AF.Square,
                                             bias=b2_sb[:, 8 + fc:9 + fc])
                    nc.vector.tensor_add(t1, t1, t2)
                    nc.scalar.activation(ampT[:, fc], t1, AF.Sqrt)
                    for bt in range(2):
                        bs = slice(bt * 128, (bt + 1) * 128)
                        # pg[0]/pg[1] share one PSUM bank: only the very first
                        # matmul may set start (a start wipes the whole bank)
                        nc.tensor.matmul(pg[bt], ampT[:, fc, bs], wgn_sb[:, fc],
                                         start=(fc == 0 and bt == 0),
                                         stop=(fc == 7))

            if _DEBUG_DUMP:
                nc.sync.dma_start(dbg_o1r, o1rT)
                nc.sync.dma_start(dbg_amp, ampT)

            scopeD.__exit__(None, None, None)
            scopeE = nc.named_scope("stageE_gate"); scopeE.__enter__()

            out_v = out_d.rearrange("(bt p) e -> bt p e", bt=2)
            for bt in range(2):
                logits = stage.tile([128, E], F32, tag="logits", bufs=2)
                if training:
                    stdn = stage.tile([128, E], F32, tag="stdn", bufs=2)
                    # softplus(z) = ln(1 + exp(z))
                    nc.scalar.activation(stdn, pg[bt][:, 128:128 + E], AF.Exp)
                    nc.vector.tensor_scalar_add(stdn, stdn, 1.0)
                    nc.scalar.activation(stdn, stdn, AF.Ln)
                    # (softplus(z)+eps0)*eps
                    nc.vector.scalar_tensor_tensor(
                        stdn, stdn, float(NOISE_EPS), eps_sb[:, bt],
                        op0=ADD, op1=MULT)
                    nc.vector.tensor_add(logits, pg[bt][:, 0:E], stdn)
                else:
                    nc.vector.tensor_copy(logits, pg[bt][:, 0:E])
                top8 = stage.tile([128, 8], F32, tag="top8", bufs=2)
                nc.vector.max(top8, logits)
                negmax = stage.tile([128, 1], F32, tag="negmax", bufs=2)
                nc.vector.tensor_scalar(negmax, top8[:, 0:1], -1.0, None,
                                        op0=MULT)
                ex = stage.tile([128, E], F32, tag="ex", bufs=2)
                nc.scalar.activation(ex, logits, AF.Exp, bias=negmax)
                msk = stage.tile([128, E], F32, tag="msk", bufs=2)
                nc.vector.tensor_scalar(msk, logits, top8[:, 2:3], None,
                                        op0=mybir.AluOpType.is_ge)
                nc.vector.tensor_mul(ex, ex, msk)
                ssum = stage.tile([128, 1], F32, tag="ssum", bufs=2)
                nc.vector.reduce_sum(out=ssum, in_=ex, axis=mybir.AxisListType.X)
                rinv = stage.tile([128, 1], F32, tag="rinv", bufs=2)
                nc.vector.reciprocal(rinv, ssum)
                gates = stage.tile([128, E], F32, tag="gates", bufs=2)
                nc.vector.tensor_scalar(gates, ex, rinv, None, op0=MULT)
                nc.sync.dma_start(out_v[bt], gates)

            scopeE.__exit__(None, None, None)

    nc.compile()
    return nc


_PROGRAM_CACHE = {}


def _get_program(training: bool):
    key = bool(training)
    if key not in _PROGRAM_CACHE:
        _PROGRAM_CACHE[key] = _build_program(key)
    return _PROGRAM_CACHE[key]


def _prep_inputs(x, fc_w, fc_b, w1, b1, w2, b2, w_gate, w_noise, eps):
    f32 = np.float32
    f16 = np.float16

    # ---- weights/constants shared by all cores ----
    ll = np.arange(1, F, dtype=np.int64)[:, None]  # l' = 1..1023
    ff = np.arange(1, F + 1, dtype=np.int64)[None, :]
    ang = 2.0 * np.pi * ((ll * ff) % L).astype(np.float64) / L
    scale = 1.0 / np.sqrt(L)
    Ch = np.empty((F, F), np.float64)
    Sh = np.empty((F, F), np.float64)
    Ch[0, :] = scale
    Ch[1:, :] = np.cos(ang) * scale
    Sh[0, :] = 0.0
    Sh[1:, :] = -np.sin(ang) * scale
    # [p(l'), kc, fc, 128]
    chs = Ch.astype(f16).reshape(8, 128, 8, 128).transpose(1, 0, 2, 3)
    shs = Sh.astype(f16).reshape(8, 128, 8, 128).transpose(1, 0, 2, 3)

    # midpoint row: C[1024, f] = (-1)^f * scale, f = p+1 within each chunk
    p = np.arange(128)
    alt = (np.where((p + 1) % 2 == 0, 1.0, -1.0) * scale).astype(f16)
    alt = alt.reshape(1, 128)

    w1r = np.asarray(w1[0], f32)
    w1i = np.asarray(w1[1], f32)
    if KARA1:
        k1m, k2m, k3m = w1r, w1r + w1i, w1i - w1r
    else:
        k1m, k2m, k3m = w1r, w1i, w1i  # k3 unused

    def tile1(M):
        return np.ascontiguousarray(
            M.reshape(8, 128, 8, 512).transpose(2, 1, 0, 3).astype(f16))

    w2r = np.asarray(w2[0], f32)
    w2i = np.asarray(w2[1], f32)
    if KARA2:
        m1m, m2m, m3m = w2r, w2r + w2i, w2i - w2r
    else:
        m1m, m2m, m3m = w2r, w2i, w2i  # m3 unused

    def tile2(M):
        return np.ascontiguousarray(
            M.reshape(32, 128, 4, 256).transpose(2, 1, 0, 3).astype(f16))

    wgn = np.zeros((F, 256), f32)
    wgn[:, 0:E] = np.asarray(w_gate, f32)
    wgn[:, 128:128 + E] = np.asarray(w_noise, f32)
    wgn = rnd11(wgn).reshape(8, 128, 256).transpose(1, 0, 2)

    b1all = np.zeros((128, 64), f32)
    b1all[:, 0:32] = np.asarray(b1[0], f32).reshape(32, 128).T
    b1all[:, 32:64] = np.asarray(b1[1], f32).reshape(32, 128).T
    b2all = np.zeros((128, 16), f32)
    b2all[:, 0:8] = np.asarray(b2[0], f32).reshape(8, 128).T
    b2all[:, 8:16] = np.asarray(b2[1], f32).reshape(8, 128).T

    common = {
        "chs": np.ascontiguousarray(chs),
        "shs": np.ascontiguousarray(shs),
        "k1": tile1(k1m),
        "k2": tile1(k2m),
        "k3": tile1(k3m),
        "m1": tile2(m1m),
        "m2": tile2(m2m),
        "m3": tile2(m3m),
        "wgn": np.ascontiguousarray(wgn),
        "altrow": alt,
        "b1all": b1all,
        "b2all": b2all,
    }

    # ---- per-core data ----
    x = np.asarray(x, f32)
    fcw = np.asarray(fc_w, f32).reshape(CH)
    eps = np.asarray(eps, f32)

    in_maps = []
    for i in range(NCORES):
        xs = x[i * BL:(i + 1) * BL]  # [256, 2048, 16]
        # xw[b, c, l] = x[b, l, c] * fc_w[c] (scale folded host-side)
        xw = xs.transpose(0, 2, 1) * fcw[None, :, None]  # [256, 16, 2048]
        xe = np.empty((BL, CH, F), f32)
        xo = np.empty((BL, CH, F), f32)
        xe[:, :, 0] = xw[:, :, 0]
        xo[:, :, 0] = 0.0
        fwd = xw[:, :, 1:1024]
        rev = xw[:, :, 2047:1024:-1]
        xe[:, :, 1:1024] = fwd + rev
        xo[:, :, 1:1024] = fwd - rev
        hm = xw[:, :, 1024].sum(axis=1)  # [256]
        # tile: [eo, b, c, l'] -> [eo, lc, p(l'), c, b]
        xeo = np.stack([xe, xo])  # [2, 256, 16, 1024]
        xeo = xeo.transpose(0, 3, 2, 1).astype(f16)  # [2, 1024, 16, 256]
        sh = dict(common)
        sh["xeo"] = np.ascontiguousarray(xeo.reshape(2, 8, 128, CH, 256))
        sh["hmrow"] = hm.astype(f16).reshape(1, 256)
        esh = eps[i * BL:(i + 1) * BL]  # [256, E]
        sh["eps"] = np.ascontiguousarray(esh.reshape(2, 128, E).transpose(1, 0, 2))
        in_maps.append(sh)
    return in_maps


def run(inputs, trace=False):
    """Returns (gates [B, E] float32, BassKernelResults)."""
    x = np.asarray(inputs["x"], np.float32)
    fc_w = np.asarray(inputs["fc_w"], np.float32)
    fc_b = np.asarray(inputs["fc_b"], np.float32)
    w1 = np.asarray(inputs["w1"], np.float32)
    b1 = np.asarray(inputs["b1"], np.float32)
    w2 = np.asarray(inputs["w2"], np.float32)
    b2 = np.asarray(inputs["b2"], np.float32)
    w_gate = np.asarray(inputs["w_gate"], np.float32)
    w_noise = np.asarray(inputs["w_noise"], np.float32)
    eps = np.asarray(inputs["eps"], np.float32)
    training = bool(int(np.asarray(inputs.get("training", 1))))

    nc = _get_program(training)
    in_maps = _prep_inputs(x, fc_w, fc_b, w1, b1, w2, b2, w_gate, w_noise, eps)
    res = run_bass_kernel_spmd(
        nc, in_maps, core_ids=list(range(NCORES)), trace=trace,
    )
    gates = np.concatenate([r["out"] for r in res.results], axis=0)
    return gates.astype(np.float32), res


def kernel(**inputs):
    gates, _ = run(inputs, trace=False)
    return gates
